# revision 28
# baseline (speedup 1.0000x reference)
"""EnergyScoreLoss Trainium2 kernel.

Math: for each element e of the [B, D] grid (flattened), with n=50 samples:
  samples_s = mean + noise_s * std,  std = sqrt(var + 1e-6)
  first   = (1/n) * sum_s |samples_s - target|
  pairsum = sum_k (2k - n + 1) * sorted(samples)_k
  energy  = first - (beta/2) * pairsum / (n(n-1)/2)
  out     = mean_e(energy)

Device formulation (per element, scale/shift-invariant tricks):
  u_s = noise_s/50 + c',  c' = (mean - target) / std / 50   (fp16)
  first   = std * sum_s |u_s|,  sum_s |u_s| = 2*sum relu(u) - sum u
  sorting u == sorting samples (std > 0), and since sum coef_k = 0 the
  shift by c' drops out of the weighted sum:
  energy  = std * (sum_s |u_s| - wsum / 49),  wsum = sum_k coef_k u_(k)

Sharding: batch across 8 cores (65536 elements each). SBUF layout: element
e -> (partition p, col c), e = p*512 + c. Samples live in 50 blocks of 512
cols (sample-major), sorted by a pruned Batcher odd-even merge network
(403 compare-exchanges, 21 rounds) using fp16 tensor_tensor min/max at the
DVE 2x perf mode. Untouched wires are ping-ponged by the (otherwise idle)
DMA engines (heaviest rounds also use the Scalar engine). The sort runs on
raw converted noise; the first term's relu tree hides its first level in
the DMA-bound input pipeline; the shift correction lands in fp32 at the
final combine.
"""

import sys

for _p in ("/opt/trn_rl_repo", "/root/.axon_site/_ro/trn_rl_repo"):
    if _p not in sys.path:
        sys.path.insert(0, _p)

import numpy as np

N_SAMPLES = 50
N_CORES = 8
B, D = 8192, 64
V = B * D // N_CORES          # elements per core
E = V // 128                  # cols per partition
EPS = 1e-6


def _oems_rounds(n_pow2, n_real):
    """Batcher odd-even merge sort, pruned to wires < n_real.
    All comparators send min to the lower wire."""
    rounds = []
    p = 1
    while p < n_pow2:
        k = p
        while k >= 1:
            pairs = []
            for j in range(k % p, n_pow2 - k, 2 * k):
                for i in range(0, min(k, n_pow2 - j - k)):
                    a, b = i + j, i + j + k
                    if (a // (p * 2)) == (b // (p * 2)) and b < n_real:
                        pairs.append((a, b))
            if pairs:
                rounds.append(pairs)
            k //= 2
        p *= 2
    return rounds


def _runs_of(pairs):
    k = pairs[0][1] - pairs[0][0]
    lefts = sorted(a for a, _ in pairs)
    runs = []
    s = prev = lefts[0]
    for x in lefts[1:]:
        if x == prev + 1:
            prev = x
        else:
            runs.append((s, prev - s + 1))
            s = prev = x
    runs.append((s, prev - s + 1))
    return k, runs


def _group_runs(runs):
    """Group equal-length runs with arithmetic-progression starts:
    (start, runlen, spacing, nruns). Then merge groups that themselves
    form an arithmetic progression of starts into super-groups
    (start, runlen, spacing, nruns, spacing2, ngroups)."""
    by_len = {}
    for s, length in runs:
        by_len.setdefault(length, []).append(s)
    groups = []
    for length, starts in sorted(by_len.items()):
        starts.sort()
        i = 0
        while i < len(starts):
            if i + 1 < len(starts):
                d = starts[i + 1] - starts[i]
                j = i + 1
                while j + 1 < len(starts) and starts[j + 1] - starts[j] == d:
                    j += 1
                groups.append((starts[i], length, d, j - i + 1))
                i = j + 1
            else:
                groups.append((starts[i], length, 1, 1))
                i += 1
    # super-group: same (runlen, spacing, nruns), starts in AP
    out = []
    by_shape = {}
    for (s0, ln, sp, nr) in groups:
        by_shape.setdefault((ln, sp, nr), []).append(s0)
    for (ln, sp, nr), starts in sorted(by_shape.items()):
        starts.sort()
        i = 0
        while i < len(starts):
            if i + 1 < len(starts):
                d2 = starts[i + 1] - starts[i]
                j = i + 1
                while j + 1 < len(starts) and starts[j + 1] - starts[j] == d2:
                    j += 1
                out.append((starts[i], ln, sp, nr, d2, j - i + 1))
                i = j + 1
            else:
                out.append((starts[i], ln, sp, nr, 1, 1))
                i += 1
    return out


def _wire_runs(wires):
    runs = []
    if not wires:
        return runs
    s = prev = wires[0]
    for x in wires[1:]:
        if x == prev + 1:
            prev = x
        else:
            runs.append((s, prev - s + 1))
            s = prev = x
    runs.append((s, prev - s + 1))
    return runs


def _build_kernel():
    import bass_rust
    import concourse.bacc as bacc
    import concourse.mybir as mybir
    import concourse.tile as tile

    f32 = mybir.dt.float32
    f16 = mybir.dt.float16
    Alu = mybir.AluOpType
    Act = mybir.ActivationFunctionType

    nc = bacc.Bacc("TRN2", target_bir_lowering=False, debug=False,
                   num_devices=N_CORES)

    noise_d = nc.declare_dram_parameter("noise", [N_SAMPLES, V], f32,
                                        isOutput=False)
    mean_d = nc.declare_dram_parameter("mean", [128, E], f32, isOutput=False)
    var_d = nc.declare_dram_parameter("variance", [128, E], f32,
                                      isOutput=False)
    target_d = nc.declare_dram_parameter("target", [128, E], f32,
                                         isOutput=False)
    out_d = nc.declare_dram_parameter("out", [1, 1], f32, isOutput=True)

    rounds = _oems_rounds(64, N_SAMPLES)

    def blk_ap(t, start, length, spacing=1, nruns=1, spacing2=1, ngroups=1):
        """AP over `ngroups` super-groups (spacing2 apart) of `nruns` runs
        (spacing apart) of `length` consecutive blocks from block `start`."""
        base = t[:]
        part_dim = list(base.ap[0])
        ap = [part_dim]
        if ngroups > 1:
            ap.append([spacing2 * E, ngroups])
        if nruns > 1:
            ap.append([spacing * E, nruns])
        ap.append([1, length * E])
        return bass_rust.AP(tensor=base.tensor, offset=start * E, ap=ap)

    def dram_rows_ap(s0, nrows):
        """noise rows [s0, s0+nrows) as [128 partitions, nrows, E]."""
        base = noise_d[:]
        return bass_rust.AP(tensor=base.tensor, offset=s0 * V,
                            ap=[[E, 128], [V, nrows], [1, E]])

    with tile.TileContext(nc) as tc:
        with (
            tc.tile_pool(name="stage", bufs=18) as stage_pool,
            tc.tile_pool(name="big", bufs=1) as big_pool,
            tc.tile_pool(name="small", bufs=1) as small_pool,
            tc.tile_pool(name="psum", bufs=1, space="PSUM") as psum_pool,
        ):
            U = big_pool.tile([128, N_SAMPLES, E], f16, tag="U")
            W = big_pool.tile([128, N_SAMPLES, E], f16, tag="W")

            mean_t = small_pool.tile([128, E], f32, tag="mean")
            var_t = small_pool.tile([128, E], f32, tag="var")
            target_t = small_pool.tile([128, E], f32, tag="target")
            std_t = small_pool.tile([128, E], f32, tag="std")
            rstd_t = small_pool.tile([128, E], f32, tag="rstd")
            diff_t = small_pool.tile([128, E], f32, tag="diff")
            c16_t = small_pool.tile([128, E], f16, tag="c16")
            relu_sum = small_pool.tile([128, E], f32, tag="relu_sum")
            wsum_t = small_pool.tile([128, E], f32, tag="wsum")
            en_t = small_pool.tile([128, E], f32, tag="en")
            part_t = small_pool.tile([128, 1], f32, tag="part")
            ones_t = small_pool.tile([128, 1], f32, tag="ones")
            eps_t = small_pool.tile([128, 1], f32, tag="eps")
            res_t = small_pool.tile([1, 1], f32, tag="res")
            ps_t = psum_pool.tile([1, 1], f32, tag="ps")

            nc.vector.memset(eps_t[:], EPS)
            nc.sync.dma_start(mean_t[:], mean_d[:])
            nc.sync.dma_start(var_t[:], var_d[:])
            nc.sync.dma_start(target_t[:], target_d[:])

            # input DMA + convert pipeline, 2 sample rows per chunk.
            # The first chunks use single-row DMAs so the pipeline's head
            # latency is half a chunk, not a full one.
            for ch in range(N_SAMPLES // 2):
                s0 = 2 * ch
                st = stage_pool.tile([128, 2, E], f32, tag="stage")
                if ch < 3:
                    nc.sync.dma_start(st[:][:, 0, :], dram_rows_ap(s0, 1))
                    nc.sync.dma_start(st[:][:, 1, :], dram_rows_ap(s0 + 1, 1))
                else:
                    nc.sync.dma_start(st[:], dram_rows_ap(s0, 2))
                nc.scalar.activation(blk_ap(W, s0, 2), st[:].rearrange(
                    "p s c -> p (s c)"), Act.Copy, scale=0.02)

            # std = sqrt(var + eps); rstd = 1/std
            nc.scalar.activation(std_t[:], var_t[:], Act.Sqrt, bias=eps_t[:])
            nc.vector.reciprocal(rstd_t[:], std_t[:])
            # c' = (mean - target) * 0.02 * rstd  -> fp16
            nc.vector.tensor_tensor(diff_t[:], mean_t[:], target_t[:],
                                    op=Alu.subtract)
            nc.vector.scalar_tensor_tensor(c16_t[:], diff_t[:], 0.02,
                                           rstd_t[:], op0=Alu.mult,
                                           op1=Alu.mult)
            c_b2 = bass_rust.AP(tensor=c16_t[:].tensor, offset=0,
                                ap=[list(c16_t[:].ap[0]), [0, 2], [1, E]])

            # first term: U[s] = relu(W[s] + c') per chunk (every 3rd relu
            # on the Scalar engine), then a grouped tree-sum over U. The
            # sort runs on the RAW converted noise in W (the shift by c'
            # cancels in the weighted sum, corrected by -diff at the end),
            # and only writes U after the tree has consumed it (DVE is
            # in-order).
            half0 = N_SAMPLES // 2
            for ch in range(N_SAMPLES // 2):
                s0 = 2 * ch
                nc.vector.tensor_tensor(U[:, s0:s0 + 2, :],
                                        W[:, s0:s0 + 2, :], c_b2, op=Alu.add)
                nc.vector.tensor_scalar_max(blk_ap(U, s0, 2),
                                            blk_ap(U, s0, 2), 0.0)
                for s in (s0, s0 + 1):
                    # hidden tree level-1: U[j] += U[j+25] once both exist
                    if s >= half0:
                        j = s - half0
                        nc.vector.tensor_tensor(blk_ap(U, j, 1),
                                                blk_ap(U, j, 1),
                                                blk_ap(U, s, 1), op=Alu.add)

            def tree_levels(t, off, cnt, out32):
                while cnt > 1:
                    half = cnt // 2
                    odd = cnt % 2
                    lo = blk_ap(t, off + odd, half)
                    hi = blk_ap(t, off + half + odd, half)
                    if cnt == 2:
                        nc.vector.tensor_tensor(out32[:], lo, hi, op=Alu.add)
                    else:
                        nc.vector.tensor_tensor(lo, lo, hi, op=Alu.add)
                    cnt = half + odd

            # relu_sum = sum_s relu(u_s); level 1 already done in the loop
            tree_levels(U, 0, N_SAMPLES // 2, relu_sum)

            # sort the raw noise: ping-pong W<->U. Untouched wires move by
            # DMA; in copy-heavy rounds (aggregate DMA bandwidth would
            # stall the next round) a slice of the copies goes to the
            # otherwise-idle Scalar engine.
            cur, oth = W, U
            for pairs in rounds:
                k, runs = _runs_of(pairs)
                groups = _group_runs(runs)
                touched = set()
                for a, b in pairs:
                    touched.add(a)
                    touched.add(b)
                for (s0, ln, sp, nr, sp2, ng) in groups:
                    lo_in = blk_ap(cur, s0, ln, sp, nr, sp2, ng)
                    hi_in = blk_ap(cur, s0 + k, ln, sp, nr, sp2, ng)
                    lo_out = blk_ap(oth, s0, ln, sp, nr, sp2, ng)
                    hi_out = blk_ap(oth, s0 + k, ln, sp, nr, sp2, ng)
                    nc.vector.tensor_tensor(lo_out, lo_in, hi_in, op=Alu.min)
                    nc.vector.tensor_tensor(hi_out, lo_in, hi_in, op=Alu.max)
                unt = sorted(set(range(N_SAMPLES)) - touched)
                heavy = len(unt) > 20
                ci = 0
                for (cs, cl) in _wire_runs(unt):
                    # split into <=3-block chunks to spread across DMA queues
                    off = 0
                    while off < cl:
                        c = min(3, cl - off)
                        if heavy and ci % 3 == 2:
                            nc.scalar.copy(blk_ap(oth, cs + off, c),
                                           blk_ap(cur, cs + off, c))
                        else:
                            nc.sync.dma_start(blk_ap(oth, cs + off, c),
                                              blk_ap(cur, cs + off, c))
                        off += c
                        ci += 1
                cur, oth = oth, cur

            # bracket = sum|u| - wsum/49 = 2*sum relu(u) - sum_j [s_j +
            # (coef_j/49) d_j], with s_j/d_j the sums/differences of the
            # symmetric sorted pairs (m_{25+j}, m_{24-j}).
            half = N_SAMPLES // 2
            hi_half = cur[:][:, half:N_SAMPLES, :]
            lo_half = cur[:][:, half - 1::-1, :]
            nc.vector.tensor_tensor(oth[:][:, 0:half, :], hi_half, lo_half,
                                    op=Alu.subtract)
            nc.vector.tensor_tensor(oth[:][:, half:N_SAMPLES, :], hi_half,
                                    lo_half, op=Alu.add)
            for j in range(half):
                coef = float(2 * (half + j) - (N_SAMPLES - 1))
                nc.vector.tensor_scalar_mul(blk_ap(oth, j, 1),
                                            blk_ap(oth, j, 1),
                                            coef / (N_SAMPLES - 1.0))
            tree_levels(oth, 0, N_SAMPLES, wsum_t)

            # The sort ran on raw noise m = u - c', so sum_j s_j = sum u -
            # 50 c'; correct with -std*50*c' = -(mean - target) = -diff.
            # energy = std * (2*relu_sum - wsum_t) - diff
            nc.vector.scalar_tensor_tensor(en_t[:], relu_sum[:], 2.0,
                                           wsum_t[:], op0=Alu.mult,
                                           op1=Alu.subtract)
            nc.vector.tensor_tensor(en_t[:], en_t[:], std_t[:], op=Alu.mult)
            nc.vector.tensor_tensor(en_t[:], en_t[:], diff_t[:],
                                    op=Alu.subtract)
            nc.vector.tensor_reduce(part_t[:], en_t[:],
                                    axis=mybir.AxisListType.X, op=Alu.add)
            nc.vector.memset(ones_t[:], 1.0)
            nc.tensor.matmul(ps_t[:], part_t[:], ones_t[:])
            nc.scalar.copy(res_t[:], ps_t[:])
            nc.sync.dma_start(out_d[:], res_t[:])

    nc.compile()
    return nc


_NC_CACHE = None


def _get_nc():
    global _NC_CACHE
    if _NC_CACHE is None:
        _NC_CACHE = _build_kernel()
    return _NC_CACHE


def kernel(mean, variance, noise, target):
    from concourse.bass_utils import run_bass_kernel_spmd

    nc = _get_nc()

    mean = np.ascontiguousarray(mean, dtype=np.float32).reshape(B * D)
    variance = np.ascontiguousarray(variance, dtype=np.float32).reshape(B * D)
    target = np.ascontiguousarray(target, dtype=np.float32).reshape(B * D)
    noise = np.ascontiguousarray(noise, dtype=np.float32).reshape(N_SAMPLES,
                                                                  B * D)

    in_maps = []
    for c in range(N_CORES):
        sl = slice(c * V, (c + 1) * V)
        in_maps.append({
            "noise": np.ascontiguousarray(noise[:, sl]),
            "mean": mean[sl].reshape(128, E),
            "variance": variance[sl].reshape(128, E),
            "target": target[sl].reshape(128, E),
        })

    res = run_bass_kernel_spmd(nc, in_maps, core_ids=list(range(N_CORES)))
    total = sum(float(res.results[c]["out"][0, 0]) for c in range(N_CORES))
    return np.float32(total / (B * D))


# revision 30
# speedup vs baseline: 1.0140x; 1.0140x over previous
"""EnergyScoreLoss Trainium2 kernel.

Math: for each element e of the [B, D] grid (flattened), with n=50 samples:
  samples_s = mean + noise_s * std,  std = sqrt(var + 1e-6)
  first   = (1/n) * sum_s |samples_s - target|
  pairsum = sum_k (2k - n + 1) * sorted(samples)_k
  energy  = first - (beta/2) * pairsum / (n(n-1)/2)
  out     = mean_e(energy)

Device formulation (per element, scale/shift-invariant tricks):
  u_s = noise_s/50 + c',  c' = (mean - target) / std / 50   (fp16)
  first   = std * sum_s |u_s|,  sum_s |u_s| = 2*sum relu(u) - sum u
  sorting u == sorting samples (std > 0), and since sum coef_k = 0 the
  shift by c' drops out of the weighted sum:
  energy  = std * (sum_s |u_s| - wsum / 49),  wsum = sum_k coef_k u_(k)

Sharding: batch across 8 cores (65536 elements each). SBUF layout: element
e -> (partition p, col c), e = p*512 + c. Samples live in 50 blocks of 512
cols (sample-major), sorted by a pruned Batcher odd-even merge network
(403 compare-exchanges, 21 rounds) using fp16 tensor_tensor min/max at the
DVE 2x perf mode. Untouched wires are ping-ponged by the (otherwise idle)
DMA engines (heaviest rounds also use the Scalar engine). The sort runs on
raw converted noise; the first term's relu tree hides its first level in
the DMA-bound input pipeline; the shift correction lands in fp32 at the
final combine.
"""

import sys

for _p in ("/opt/trn_rl_repo", "/root/.axon_site/_ro/trn_rl_repo"):
    if _p not in sys.path:
        sys.path.insert(0, _p)

import numpy as np

N_SAMPLES = 50
N_CORES = 8
B, D = 8192, 64
V = B * D // N_CORES          # elements per core
E = V // 128                  # cols per partition
EPS = 1e-6


def _oems_rounds(n_pow2, n_real):
    """Batcher odd-even merge sort, pruned to wires < n_real.
    All comparators send min to the lower wire."""
    rounds = []
    p = 1
    while p < n_pow2:
        k = p
        while k >= 1:
            pairs = []
            for j in range(k % p, n_pow2 - k, 2 * k):
                for i in range(0, min(k, n_pow2 - j - k)):
                    a, b = i + j, i + j + k
                    if (a // (p * 2)) == (b // (p * 2)) and b < n_real:
                        pairs.append((a, b))
            if pairs:
                rounds.append(pairs)
            k //= 2
        p *= 2
    return rounds


def _runs_of(pairs):
    k = pairs[0][1] - pairs[0][0]
    lefts = sorted(a for a, _ in pairs)
    runs = []
    s = prev = lefts[0]
    for x in lefts[1:]:
        if x == prev + 1:
            prev = x
        else:
            runs.append((s, prev - s + 1))
            s = prev = x
    runs.append((s, prev - s + 1))
    return k, runs


def _group_runs(runs):
    """Group equal-length runs with arithmetic-progression starts:
    (start, runlen, spacing, nruns). Then merge groups that themselves
    form an arithmetic progression of starts into super-groups
    (start, runlen, spacing, nruns, spacing2, ngroups)."""
    by_len = {}
    for s, length in runs:
        by_len.setdefault(length, []).append(s)
    groups = []
    for length, starts in sorted(by_len.items()):
        starts.sort()
        i = 0
        while i < len(starts):
            if i + 1 < len(starts):
                d = starts[i + 1] - starts[i]
                j = i + 1
                while j + 1 < len(starts) and starts[j + 1] - starts[j] == d:
                    j += 1
                groups.append((starts[i], length, d, j - i + 1))
                i = j + 1
            else:
                groups.append((starts[i], length, 1, 1))
                i += 1
    # super-group: same (runlen, spacing, nruns), starts in AP
    out = []
    by_shape = {}
    for (s0, ln, sp, nr) in groups:
        by_shape.setdefault((ln, sp, nr), []).append(s0)
    for (ln, sp, nr), starts in sorted(by_shape.items()):
        starts.sort()
        i = 0
        while i < len(starts):
            if i + 1 < len(starts):
                d2 = starts[i + 1] - starts[i]
                j = i + 1
                while j + 1 < len(starts) and starts[j + 1] - starts[j] == d2:
                    j += 1
                out.append((starts[i], ln, sp, nr, d2, j - i + 1))
                i = j + 1
            else:
                out.append((starts[i], ln, sp, nr, 1, 1))
                i += 1
    return out


def _wire_runs(wires):
    runs = []
    if not wires:
        return runs
    s = prev = wires[0]
    for x in wires[1:]:
        if x == prev + 1:
            prev = x
        else:
            runs.append((s, prev - s + 1))
            s = prev = x
    runs.append((s, prev - s + 1))
    return runs


def _build_kernel():
    import bass_rust
    import concourse.bacc as bacc
    import concourse.mybir as mybir
    import concourse.tile as tile

    f32 = mybir.dt.float32
    f16 = mybir.dt.float16
    Alu = mybir.AluOpType
    Act = mybir.ActivationFunctionType

    nc = bacc.Bacc("TRN2", target_bir_lowering=False, debug=False,
                   num_devices=N_CORES)

    noise_d = nc.declare_dram_parameter("noise", [N_SAMPLES, V], f32,
                                        isOutput=False)
    mean_d = nc.declare_dram_parameter("mean", [128, E], f32, isOutput=False)
    var_d = nc.declare_dram_parameter("variance", [128, E], f32,
                                      isOutput=False)
    target_d = nc.declare_dram_parameter("target", [128, E], f32,
                                         isOutput=False)
    out_d = nc.declare_dram_parameter("out", [1, 1], f32, isOutput=True)

    rounds = _oems_rounds(64, N_SAMPLES)

    def blk_ap(t, start, length, spacing=1, nruns=1, spacing2=1, ngroups=1):
        """AP over `ngroups` super-groups (spacing2 apart) of `nruns` runs
        (spacing apart) of `length` consecutive blocks from block `start`."""
        base = t[:]
        part_dim = list(base.ap[0])
        ap = [part_dim]
        if ngroups > 1:
            ap.append([spacing2 * E, ngroups])
        if nruns > 1:
            ap.append([spacing * E, nruns])
        ap.append([1, length * E])
        return bass_rust.AP(tensor=base.tensor, offset=start * E, ap=ap)

    def dram_rows_ap(s0, nrows):
        """noise rows [s0, s0+nrows) as [128 partitions, nrows, E]."""
        base = noise_d[:]
        return bass_rust.AP(tensor=base.tensor, offset=s0 * V,
                            ap=[[E, 128], [V, nrows], [1, E]])

    with tile.TileContext(nc) as tc:
        with (
            tc.tile_pool(name="stage", bufs=18) as stage_pool,
            tc.tile_pool(name="big", bufs=1) as big_pool,
            tc.tile_pool(name="small", bufs=1) as small_pool,
            tc.tile_pool(name="psum", bufs=1, space="PSUM") as psum_pool,
        ):
            U = big_pool.tile([128, N_SAMPLES, E], f16, tag="U")
            W = big_pool.tile([128, N_SAMPLES, E], f16, tag="W")

            mean_t = small_pool.tile([128, E], f32, tag="mean")
            var_t = small_pool.tile([128, E], f32, tag="var")
            target_t = small_pool.tile([128, E], f32, tag="target")
            std_t = small_pool.tile([128, E], f32, tag="std")
            rstd_t = small_pool.tile([128, E], f32, tag="rstd")
            diff_t = small_pool.tile([128, E], f32, tag="diff")
            c16_t = small_pool.tile([128, E], f16, tag="c16")
            relu_sum = small_pool.tile([128, E], f32, tag="relu_sum")
            wsum_t = small_pool.tile([128, E], f32, tag="wsum")
            en_t = small_pool.tile([128, E], f32, tag="en")
            part_t = small_pool.tile([128, 1], f32, tag="part")
            ones_t = small_pool.tile([128, 1], f32, tag="ones")
            eps_t = small_pool.tile([128, 1], f32, tag="eps")
            res_t = small_pool.tile([1, 1], f32, tag="res")
            ps_t = psum_pool.tile([1, 1], f32, tag="ps")

            nc.vector.memset(eps_t[:], EPS)
            nc.sync.dma_start(mean_t[:], mean_d[:])
            nc.sync.dma_start(var_t[:], var_d[:])
            nc.sync.dma_start(target_t[:], target_d[:])

            # input DMA + convert pipeline, 2 sample rows per chunk.
            # The first chunks use single-row DMAs so the pipeline's head
            # latency is half a chunk, not a full one.
            for ch in range(N_SAMPLES // 2):
                s0 = 2 * ch
                st = stage_pool.tile([128, 2, E], f32, tag="stage")
                if ch < 3:
                    nc.sync.dma_start(st[:][:, 0, :], dram_rows_ap(s0, 1))
                    nc.sync.dma_start(st[:][:, 1, :], dram_rows_ap(s0 + 1, 1))
                else:
                    nc.sync.dma_start(st[:], dram_rows_ap(s0, 2))
                nc.scalar.activation(blk_ap(W, s0, 2), st[:].rearrange(
                    "p s c -> p (s c)"), Act.Copy, scale=0.02)

            # std = sqrt(var + eps); rstd = 1/std
            nc.scalar.activation(std_t[:], var_t[:], Act.Sqrt, bias=eps_t[:])
            nc.vector.reciprocal(rstd_t[:], std_t[:])
            # negc = -c' = (target - mean) * 0.02 * rstd  -> fp16
            nc.vector.tensor_tensor(diff_t[:], mean_t[:], target_t[:],
                                    op=Alu.subtract)
            nc.vector.scalar_tensor_tensor(c16_t[:], diff_t[:], -0.02,
                                           rstd_t[:], op0=Alu.mult,
                                           op1=Alu.mult)
            c_b2 = bass_rust.AP(tensor=c16_t[:].tensor, offset=0,
                                ap=[list(c16_t[:].ap[0]), [0, 2], [1, E]])

            # first term: relu(w + c') = max(w, -c') + c', so a single
            # tensor_tensor max per chunk, then a grouped tree-sum over U;
            # the +50c' correction lands in exact fp32 at the final
            # combine. The sort runs on the RAW converted noise in W (the
            # shift by c' cancels in the weighted sum too), and only
            # writes U after the tree has consumed it (DVE is in-order).
            half0 = N_SAMPLES // 2
            for ch in range(N_SAMPLES // 2):
                s0 = 2 * ch
                nc.vector.tensor_tensor(U[:, s0:s0 + 2, :],
                                        W[:, s0:s0 + 2, :], c_b2, op=Alu.max)
                for s in (s0, s0 + 1):
                    # hidden tree level-1: U[j] += U[j+25] once both exist
                    if s >= half0:
                        j = s - half0
                        nc.vector.tensor_tensor(blk_ap(U, j, 1),
                                                blk_ap(U, j, 1),
                                                blk_ap(U, s, 1), op=Alu.add)

            def tree_levels(t, off, cnt, out32):
                while cnt > 1:
                    half = cnt // 2
                    odd = cnt % 2
                    lo = blk_ap(t, off + odd, half)
                    hi = blk_ap(t, off + half + odd, half)
                    if cnt == 2:
                        nc.vector.tensor_tensor(out32[:], lo, hi, op=Alu.add)
                    else:
                        nc.vector.tensor_tensor(lo, lo, hi, op=Alu.add)
                    cnt = half + odd

            # relu_sum = sum_s relu(u_s); level 1 already done in the loop
            tree_levels(U, 0, N_SAMPLES // 2, relu_sum)

            # sort the raw noise: ping-pong W<->U. Untouched wires move by
            # DMA; in copy-heavy rounds (aggregate DMA bandwidth would
            # stall the next round) a slice of the copies goes to the
            # otherwise-idle Scalar engine.
            cur, oth = W, U
            for pairs in rounds:
                k, runs = _runs_of(pairs)
                groups = _group_runs(runs)
                touched = set()
                for a, b in pairs:
                    touched.add(a)
                    touched.add(b)
                for (s0, ln, sp, nr, sp2, ng) in groups:
                    lo_in = blk_ap(cur, s0, ln, sp, nr, sp2, ng)
                    hi_in = blk_ap(cur, s0 + k, ln, sp, nr, sp2, ng)
                    lo_out = blk_ap(oth, s0, ln, sp, nr, sp2, ng)
                    hi_out = blk_ap(oth, s0 + k, ln, sp, nr, sp2, ng)
                    nc.vector.tensor_tensor(lo_out, lo_in, hi_in, op=Alu.min)
                    nc.vector.tensor_tensor(hi_out, lo_in, hi_in, op=Alu.max)
                unt = sorted(set(range(N_SAMPLES)) - touched)
                heavy = len(unt) > 20
                ci = 0
                for (cs, cl) in _wire_runs(unt):
                    # split into <=3-block chunks to spread across DMA queues
                    off = 0
                    while off < cl:
                        c = min(3, cl - off)
                        if heavy and ci % 3 == 2:
                            nc.scalar.copy(blk_ap(oth, cs + off, c),
                                           blk_ap(cur, cs + off, c))
                        else:
                            nc.sync.dma_start(blk_ap(oth, cs + off, c),
                                              blk_ap(cur, cs + off, c))
                        off += c
                        ci += 1
                cur, oth = oth, cur

            # bracket = sum|u| - wsum/49 = 2*sum relu(u) - sum_j [s_j +
            # (coef_j/49) d_j], with s_j/d_j the sums/differences of the
            # symmetric sorted pairs (m_{25+j}, m_{24-j}).
            half = N_SAMPLES // 2
            hi_half = cur[:][:, half:N_SAMPLES, :]
            lo_half = cur[:][:, half - 1::-1, :]
            nc.vector.tensor_tensor(oth[:][:, 0:half, :], hi_half, lo_half,
                                    op=Alu.subtract)
            nc.vector.tensor_tensor(oth[:][:, half:N_SAMPLES, :], hi_half,
                                    lo_half, op=Alu.add)
            for j in range(half):
                coef = float(2 * (half + j) - (N_SAMPLES - 1))
                nc.vector.tensor_scalar_mul(blk_ap(oth, j, 1),
                                            blk_ap(oth, j, 1),
                                            coef / (N_SAMPLES - 1.0))
            tree_levels(oth, 0, N_SAMPLES, wsum_t)

            # relu_sum holds M = sum_s max(w_s, -c'), and the sort ran on
            # raw noise w = u - c'. bracket = 2(M + 50c') - (sum u) -
            # wsum/49 = 2M - wsum_t + 50c', so:
            # energy = std * (2M - wsum_t) + std*50*c' = ... + diff
            nc.vector.scalar_tensor_tensor(en_t[:], relu_sum[:], 2.0,
                                           wsum_t[:], op0=Alu.mult,
                                           op1=Alu.subtract)
            nc.vector.tensor_tensor(en_t[:], en_t[:], std_t[:], op=Alu.mult)
            nc.vector.tensor_tensor(en_t[:], en_t[:], diff_t[:], op=Alu.add)
            nc.vector.tensor_reduce(part_t[:], en_t[:],
                                    axis=mybir.AxisListType.X, op=Alu.add)
            nc.vector.memset(ones_t[:], 1.0)
            nc.tensor.matmul(ps_t[:], part_t[:], ones_t[:])
            nc.scalar.copy(res_t[:], ps_t[:])
            nc.sync.dma_start(out_d[:], res_t[:])

    nc.compile()
    return nc


_NC_CACHE = None


def _get_nc():
    global _NC_CACHE
    if _NC_CACHE is None:
        _NC_CACHE = _build_kernel()
    return _NC_CACHE


def kernel(mean, variance, noise, target):
    from concourse.bass_utils import run_bass_kernel_spmd

    nc = _get_nc()

    mean = np.ascontiguousarray(mean, dtype=np.float32).reshape(B * D)
    variance = np.ascontiguousarray(variance, dtype=np.float32).reshape(B * D)
    target = np.ascontiguousarray(target, dtype=np.float32).reshape(B * D)
    noise = np.ascontiguousarray(noise, dtype=np.float32).reshape(N_SAMPLES,
                                                                  B * D)

    in_maps = []
    for c in range(N_CORES):
        sl = slice(c * V, (c + 1) * V)
        in_maps.append({
            "noise": np.ascontiguousarray(noise[:, sl]),
            "mean": mean[sl].reshape(128, E),
            "variance": variance[sl].reshape(128, E),
            "target": target[sl].reshape(128, E),
        })

    res = run_bass_kernel_spmd(nc, in_maps, core_ids=list(range(N_CORES)))
    total = sum(float(res.results[c]["out"][0, 0]) for c in range(N_CORES))
    return np.float32(total / (B * D))


# revision 31
# speedup vs baseline: 1.0169x; 1.0029x over previous
"""EnergyScoreLoss Trainium2 kernel.

Math: for each element e of the [B, D] grid (flattened), with n=50 samples:
  samples_s = mean + noise_s * std,  std = sqrt(var + 1e-6)
  first   = (1/n) * sum_s |samples_s - target|
  pairsum = sum_k (2k - n + 1) * sorted(samples)_k
  energy  = first - (beta/2) * pairsum / (n(n-1)/2)
  out     = mean_e(energy)

Device formulation (per element, scale/shift-invariant tricks):
  u_s = noise_s/50 + c',  c' = (mean - target) / std / 50   (fp16)
  first   = std * sum_s |u_s|,  sum_s |u_s| = 2*sum relu(u) - sum u
  sorting u == sorting samples (std > 0), and since sum coef_k = 0 the
  shift by c' drops out of the weighted sum:
  energy  = std * (sum_s |u_s| - wsum / 49),  wsum = sum_k coef_k u_(k)

Sharding: batch across 8 cores (65536 elements each). SBUF layout: element
e -> (partition p, col c), e = p*512 + c. Samples live in 50 blocks of 512
cols (sample-major), sorted by a pruned Batcher odd-even merge network
(403 compare-exchanges, 21 rounds) using fp16 tensor_tensor min/max at the
DVE 2x perf mode. Untouched wires are ping-ponged by the (otherwise idle)
DMA engines (heaviest rounds also use the Scalar engine). The sort runs on
raw converted noise; the first term's relu tree hides its first level in
the DMA-bound input pipeline; the shift correction lands in fp32 at the
final combine.
"""

import sys

for _p in ("/opt/trn_rl_repo", "/root/.axon_site/_ro/trn_rl_repo"):
    if _p not in sys.path:
        sys.path.insert(0, _p)

import numpy as np

N_SAMPLES = 50
N_CORES = 8
B, D = 8192, 64
V = B * D // N_CORES          # elements per core
E = V // 128                  # cols per partition
EPS = 1e-6


def _oems_rounds(n_pow2, n_real):
    """Batcher odd-even merge sort, pruned to wires < n_real.
    All comparators send min to the lower wire."""
    rounds = []
    p = 1
    while p < n_pow2:
        k = p
        while k >= 1:
            pairs = []
            for j in range(k % p, n_pow2 - k, 2 * k):
                for i in range(0, min(k, n_pow2 - j - k)):
                    a, b = i + j, i + j + k
                    if (a // (p * 2)) == (b // (p * 2)) and b < n_real:
                        pairs.append((a, b))
            if pairs:
                rounds.append(pairs)
            k //= 2
        p *= 2
    return rounds


def _runs_of(pairs):
    k = pairs[0][1] - pairs[0][0]
    lefts = sorted(a for a, _ in pairs)
    runs = []
    s = prev = lefts[0]
    for x in lefts[1:]:
        if x == prev + 1:
            prev = x
        else:
            runs.append((s, prev - s + 1))
            s = prev = x
    runs.append((s, prev - s + 1))
    return k, runs


def _group_runs(runs):
    """Group equal-length runs with arithmetic-progression starts:
    (start, runlen, spacing, nruns). Then merge groups that themselves
    form an arithmetic progression of starts into super-groups
    (start, runlen, spacing, nruns, spacing2, ngroups)."""
    by_len = {}
    for s, length in runs:
        by_len.setdefault(length, []).append(s)
    groups = []
    for length, starts in sorted(by_len.items()):
        starts.sort()
        i = 0
        while i < len(starts):
            if i + 1 < len(starts):
                d = starts[i + 1] - starts[i]
                j = i + 1
                while j + 1 < len(starts) and starts[j + 1] - starts[j] == d:
                    j += 1
                groups.append((starts[i], length, d, j - i + 1))
                i = j + 1
            else:
                groups.append((starts[i], length, 1, 1))
                i += 1
    # super-group: same (runlen, spacing, nruns), starts in AP
    out = []
    by_shape = {}
    for (s0, ln, sp, nr) in groups:
        by_shape.setdefault((ln, sp, nr), []).append(s0)
    for (ln, sp, nr), starts in sorted(by_shape.items()):
        starts.sort()
        i = 0
        while i < len(starts):
            if i + 1 < len(starts):
                d2 = starts[i + 1] - starts[i]
                j = i + 1
                while j + 1 < len(starts) and starts[j + 1] - starts[j] == d2:
                    j += 1
                out.append((starts[i], ln, sp, nr, d2, j - i + 1))
                i = j + 1
            else:
                out.append((starts[i], ln, sp, nr, 1, 1))
                i += 1
    return out


def _wire_runs(wires):
    runs = []
    if not wires:
        return runs
    s = prev = wires[0]
    for x in wires[1:]:
        if x == prev + 1:
            prev = x
        else:
            runs.append((s, prev - s + 1))
            s = prev = x
    runs.append((s, prev - s + 1))
    return runs


def _build_kernel():
    import bass_rust
    import concourse.bacc as bacc
    import concourse.mybir as mybir
    import concourse.tile as tile

    f32 = mybir.dt.float32
    f16 = mybir.dt.float16
    Alu = mybir.AluOpType
    Act = mybir.ActivationFunctionType

    nc = bacc.Bacc("TRN2", target_bir_lowering=False, debug=False,
                   num_devices=N_CORES)

    noise_d = nc.declare_dram_parameter("noise", [N_SAMPLES, V], f32,
                                        isOutput=False)
    mean_d = nc.declare_dram_parameter("mean", [128, E], f32, isOutput=False)
    var_d = nc.declare_dram_parameter("variance", [128, E], f32,
                                      isOutput=False)
    target_d = nc.declare_dram_parameter("target", [128, E], f32,
                                         isOutput=False)
    out_d = nc.declare_dram_parameter("out", [1, 1], f32, isOutput=True)

    rounds = _oems_rounds(64, N_SAMPLES)

    def blk_ap(t, start, length, spacing=1, nruns=1, spacing2=1, ngroups=1):
        """AP over `ngroups` super-groups (spacing2 apart) of `nruns` runs
        (spacing apart) of `length` consecutive blocks from block `start`."""
        base = t[:]
        part_dim = list(base.ap[0])
        ap = [part_dim]
        if ngroups > 1:
            ap.append([spacing2 * E, ngroups])
        if nruns > 1:
            ap.append([spacing * E, nruns])
        ap.append([1, length * E])
        return bass_rust.AP(tensor=base.tensor, offset=start * E, ap=ap)

    def dram_rows_ap(s0, nrows):
        """noise rows [s0, s0+nrows) as [128 partitions, nrows, E]."""
        base = noise_d[:]
        return bass_rust.AP(tensor=base.tensor, offset=s0 * V,
                            ap=[[E, 128], [V, nrows], [1, E]])

    with tile.TileContext(nc) as tc:
        with (
            tc.tile_pool(name="stage", bufs=18) as stage_pool,
            tc.tile_pool(name="big", bufs=1) as big_pool,
            tc.tile_pool(name="small", bufs=1) as small_pool,
            tc.tile_pool(name="psum", bufs=1, space="PSUM") as psum_pool,
        ):
            U = big_pool.tile([128, N_SAMPLES, E], f16, tag="U")
            W = big_pool.tile([128, N_SAMPLES, E], f16, tag="W")

            mean_t = small_pool.tile([128, E], f32, tag="mean")
            var_t = small_pool.tile([128, E], f32, tag="var")
            target_t = small_pool.tile([128, E], f32, tag="target")
            std_t = small_pool.tile([128, E], f32, tag="std")
            rstd_t = small_pool.tile([128, E], f32, tag="rstd")
            diff_t = small_pool.tile([128, E], f32, tag="diff")
            c16_t = small_pool.tile([128, E], f16, tag="c16")
            relu_sum = small_pool.tile([128, E], f32, tag="relu_sum")
            wsum_t = small_pool.tile([128, E], f32, tag="wsum")
            en_t = small_pool.tile([128, E], f32, tag="en")
            part_t = small_pool.tile([128, 1], f32, tag="part")
            ones_t = small_pool.tile([128, 1], f32, tag="ones")
            eps_t = small_pool.tile([128, 1], f32, tag="eps")
            res_t = small_pool.tile([1, 1], f32, tag="res")
            ps_t = psum_pool.tile([1, 1], f32, tag="ps")

            nc.vector.memset(eps_t[:], EPS)
            nc.sync.dma_start(mean_t[:], mean_d[:])
            nc.sync.dma_start(var_t[:], var_d[:])
            nc.sync.dma_start(target_t[:], target_d[:])

            # input DMA + convert pipeline, 2 sample rows per chunk.
            # The first chunks use single-row DMAs so the pipeline's head
            # latency is half a chunk, not a full one.
            for ch in range(N_SAMPLES // 2):
                s0 = 2 * ch
                st = stage_pool.tile([128, 2, E], f32, tag="stage")
                if ch < 3:
                    nc.sync.dma_start(st[:][:, 0, :], dram_rows_ap(s0, 1))
                    nc.sync.dma_start(st[:][:, 1, :], dram_rows_ap(s0 + 1, 1))
                else:
                    nc.sync.dma_start(st[:], dram_rows_ap(s0, 2))
                nc.scalar.activation(blk_ap(W, s0, 2), st[:].rearrange(
                    "p s c -> p (s c)"), Act.Copy, scale=0.02)

            # std = sqrt(var + eps); rstd = 1/std
            nc.scalar.activation(std_t[:], var_t[:], Act.Sqrt, bias=eps_t[:])
            nc.vector.reciprocal(rstd_t[:], std_t[:])
            # negc = -c' = (target - mean) * 0.02 * rstd  -> fp16
            nc.vector.tensor_tensor(diff_t[:], mean_t[:], target_t[:],
                                    op=Alu.subtract)
            nc.vector.scalar_tensor_tensor(c16_t[:], diff_t[:], -0.02,
                                           rstd_t[:], op0=Alu.mult,
                                           op1=Alu.mult)
            c_b2 = bass_rust.AP(tensor=c16_t[:].tensor, offset=0,
                                ap=[list(c16_t[:].ap[0]), [0, 2], [1, E]])

            # first term: relu(w + c') = max(w, -c') + c', so a single
            # tensor_tensor max per chunk, then a grouped tree-sum over U;
            # the +50c' correction lands in exact fp32 at the final
            # combine. The sort runs on the RAW converted noise in W (the
            # shift by c' cancels in the weighted sum too), and only
            # writes U after the tree has consumed it (DVE is in-order).
            for ch in range(N_SAMPLES // 2):
                s0 = 2 * ch
                nc.vector.tensor_tensor(U[:, s0:s0 + 2, :],
                                        W[:, s0:s0 + 2, :], c_b2, op=Alu.max)
                # hidden tree level-1 within the chunk: U[s0] += U[s0+1]
                nc.vector.tensor_tensor(blk_ap(U, s0, 1), blk_ap(U, s0, 1),
                                        blk_ap(U, s0 + 1, 1), op=Alu.add)

            def tree_levels(t, off, cnt, out32, stride=1):
                # blocks at off + stride*i for i in [0, cnt)
                while cnt > 1:
                    half = cnt // 2
                    odd = cnt % 2
                    lo = blk_ap(t, off + stride * odd, 1, stride, half)
                    hi = blk_ap(t, off + stride * (half + odd), 1, stride,
                                half)
                    if cnt == 2:
                        nc.vector.tensor_tensor(out32[:], lo, hi, op=Alu.add)
                    else:
                        nc.vector.tensor_tensor(lo, lo, hi, op=Alu.add)
                    cnt = half + odd

            # relu_sum = sum_s relu(u_s); level 1 already done in the loop,
            # partials live at even blocks of U
            tree_levels(U, 0, N_SAMPLES // 2, relu_sum, stride=2)

            # sort the raw noise: ping-pong W<->U. Untouched wires move by
            # DMA; in copy-heavy rounds (aggregate DMA bandwidth would
            # stall the next round) a slice of the copies goes to the
            # otherwise-idle Scalar engine.
            cur, oth = W, U
            for pairs in rounds:
                k, runs = _runs_of(pairs)
                groups = _group_runs(runs)
                touched = set()
                for a, b in pairs:
                    touched.add(a)
                    touched.add(b)
                for (s0, ln, sp, nr, sp2, ng) in groups:
                    lo_in = blk_ap(cur, s0, ln, sp, nr, sp2, ng)
                    hi_in = blk_ap(cur, s0 + k, ln, sp, nr, sp2, ng)
                    lo_out = blk_ap(oth, s0, ln, sp, nr, sp2, ng)
                    hi_out = blk_ap(oth, s0 + k, ln, sp, nr, sp2, ng)
                    nc.vector.tensor_tensor(lo_out, lo_in, hi_in, op=Alu.min)
                    nc.vector.tensor_tensor(hi_out, lo_in, hi_in, op=Alu.max)
                unt = sorted(set(range(N_SAMPLES)) - touched)
                heavy = len(unt) > 20
                ci = 0
                for (cs, cl) in _wire_runs(unt):
                    # split into <=3-block chunks to spread across DMA queues
                    off = 0
                    while off < cl:
                        c = min(3, cl - off)
                        if heavy and ci % 3 == 2:
                            nc.scalar.copy(blk_ap(oth, cs + off, c),
                                           blk_ap(cur, cs + off, c))
                        else:
                            nc.sync.dma_start(blk_ap(oth, cs + off, c),
                                              blk_ap(cur, cs + off, c))
                        off += c
                        ci += 1
                cur, oth = oth, cur

            # bracket = sum|u| - wsum/49 = 2*sum relu(u) - sum_j [s_j +
            # (coef_j/49) d_j], with s_j/d_j the sums/differences of the
            # symmetric sorted pairs (m_{25+j}, m_{24-j}).
            half = N_SAMPLES // 2
            hi_half = cur[:][:, half:N_SAMPLES, :]
            lo_half = cur[:][:, half - 1::-1, :]
            nc.vector.tensor_tensor(oth[:][:, 0:half, :], hi_half, lo_half,
                                    op=Alu.subtract)
            nc.vector.tensor_tensor(oth[:][:, half:N_SAMPLES, :], hi_half,
                                    lo_half, op=Alu.add)
            for j in range(half):
                coef = float(2 * (half + j) - (N_SAMPLES - 1))
                nc.vector.tensor_scalar_mul(blk_ap(oth, j, 1),
                                            blk_ap(oth, j, 1),
                                            coef / (N_SAMPLES - 1.0))
            tree_levels(oth, 0, N_SAMPLES, wsum_t)

            # relu_sum holds M = sum_s max(w_s, -c'), and the sort ran on
            # raw noise w = u - c'. bracket = 2(M + 50c') - (sum u) -
            # wsum/49 = 2M - wsum_t + 50c', so:
            # energy = std * (2M - wsum_t) + std*50*c' = ... + diff
            nc.vector.scalar_tensor_tensor(en_t[:], relu_sum[:], 2.0,
                                           wsum_t[:], op0=Alu.mult,
                                           op1=Alu.subtract)
            nc.vector.tensor_tensor(en_t[:], en_t[:], std_t[:], op=Alu.mult)
            nc.vector.tensor_tensor(en_t[:], en_t[:], diff_t[:], op=Alu.add)
            nc.vector.tensor_reduce(part_t[:], en_t[:],
                                    axis=mybir.AxisListType.X, op=Alu.add)
            nc.vector.memset(ones_t[:], 1.0)
            nc.tensor.matmul(ps_t[:], part_t[:], ones_t[:])
            nc.scalar.copy(res_t[:], ps_t[:])
            nc.sync.dma_start(out_d[:], res_t[:])

    nc.compile()
    return nc


_NC_CACHE = None


def _get_nc():
    global _NC_CACHE
    if _NC_CACHE is None:
        _NC_CACHE = _build_kernel()
    return _NC_CACHE


def kernel(mean, variance, noise, target):
    from concourse.bass_utils import run_bass_kernel_spmd

    nc = _get_nc()

    mean = np.ascontiguousarray(mean, dtype=np.float32).reshape(B * D)
    variance = np.ascontiguousarray(variance, dtype=np.float32).reshape(B * D)
    target = np.ascontiguousarray(target, dtype=np.float32).reshape(B * D)
    noise = np.ascontiguousarray(noise, dtype=np.float32).reshape(N_SAMPLES,
                                                                  B * D)

    in_maps = []
    for c in range(N_CORES):
        sl = slice(c * V, (c + 1) * V)
        in_maps.append({
            "noise": np.ascontiguousarray(noise[:, sl]),
            "mean": mean[sl].reshape(128, E),
            "variance": variance[sl].reshape(128, E),
            "target": target[sl].reshape(128, E),
        })

    res = run_bass_kernel_spmd(nc, in_maps, core_ids=list(range(N_CORES)))
    total = sum(float(res.results[c]["out"][0, 0]) for c in range(N_CORES))
    return np.float32(total / (B * D))
